# revision 5
# baseline (speedup 1.0000x reference)
"""GAT (2-layer, PyG-style) on 8 Trainium2 NeuronCores via Bass/Tile.

Strategy (dst-sharded, degree-sorted tiles, big-batch dma_gather):
- Nodes sharded by dst across 8 cores (12500 each). Per core, dsts are
  degree-sorted and grouped into 98 tiles of 128 (partition = dst).
- Per tile, column 0 gathers the dst's own table row (serving both the
  self-loop edge and the per-partition al_dst values); remaining columns
  hold in-edges, padded to the tile max degree.
- Layer tables are 4-node-packed rows (<=25088 rows, int16 dma_gather
  indices); a 4-way one-hot select on DVE picks the node within the row.
- Gathers run 4096 indices per call round-robin over the 4 SWDGE queues
  (32 columns per chunk; chunks span tile boundaries).
- Pad slots index dedicated poison rows (al = -1e30) so exp() kills
  them; no mask arrays needed.
- Segment softmax: no max-subtraction needed (logits are O(1)); the
  denominator divides the aggregated numerator once per dst row.
- b1/b2 folded into the h-columns of the tables (alpha sums to 1).
- x is pre-transposed on the host so the node phase is matmul-only
  (no PE transposes); layer-1 table writes are 640 B contiguous.
- Layer-2 per-node features (4 values) travel via AllGather of the
  per-core shards in core-local permuted order.
"""

import numpy as np
import ml_dtypes

BF16 = ml_dtypes.bfloat16

N = 100_000
E = 3_200_000
IN = 128
H1, C1 = 8, 8
HID = H1 * C1          # 64
OUT = 2
NEG = 0.2
NCORES = 8
ND = N // NCORES       # dsts per core: 12500
NT = 98                # tiles per core (98*128 = 12544)
PT = NT * 128          # padded dst slots per core: 12544
NPAD = 100_352         # x padded to 784*128
NITER = NPAD // 512    # node-phase iterations (512 nodes each): 196
CPC = 8                # gather-chunk columns (1024 slots per call)
T1R = NPAD // 4        # 25088 4-pack rows for layer-1 table
T1W = 384              # bf16 elems per table1 row (768 B); 4 x 80 used
T2R = (PT * NCORES) // 4   # 25088
T2W = 128              # bf16 elems per table2 row (256 B); 4 x 4 used
PR1 = NPAD // 4 - 88   # first poison 4-pack row (nodes 100000..100351)
PR2 = T2R - 1          # poison row in table2 (always unused slots)
BIG = -1e30


def _wrap_idx(flat):
    """int16 index array -> [128, n/16] wrapped-in-16-partitions, x8."""
    n = flat.shape[0]
    assert n % 16 == 0
    w = flat.reshape(n // 16, 16).T            # [16, n/16]
    return np.tile(w, (8, 1)).astype(np.int16)  # [128, n/16]


def _plan(src, dst):
    """Host-side index planning. Returns per-core slot arrays + schedule."""
    core = dst // ND
    dloc = dst % ND

    per_core = []
    for c in range(NCORES):
        m = core == c
        s_c = src[m]
        d_c = dloc[m]
        deg = np.bincount(d_c, minlength=ND)  # in-edges, no self loop yet
        order = np.argsort(-deg, kind="stable")  # degree-desc permutation
        perm = np.full(PT, -1, dtype=np.int64)
        perm[:ND] = order
        degp = np.zeros(PT, dtype=np.int64)
        degp[:ND] = deg[order]
        sort_by_d = np.argsort(d_c, kind="stable")
        s_sorted = s_c[sort_by_d]
        starts = np.zeros(ND + 1, dtype=np.int64)
        np.cumsum(deg, out=starts[1:])
        per_core.append(dict(perm=perm, degp=degp, s_sorted=s_sorted, starts=starts))

    # common K_t schedule: columns per tile = 1 (self/dst col) + max in-degree
    K = np.zeros(NT, dtype=np.int64)
    for t in range(NT):
        mx = 0
        for c in range(NCORES):
            d = per_core[c]["degp"][t * 128 : (t + 1) * 128]
            mx = max(mx, int(d.max()) if d.size else 0)
        K[t] = mx + 1
    ncols = int(K.sum())
    nchunks = (ncols + CPC - 1) // CPC
    ncols_pad = nchunks * CPC

    col0 = np.zeros(NT, dtype=np.int64)
    pos = 0
    for t in range(NT):
        col0[t] = pos
        pos += K[t]

    datas = []
    for c in range(NCORES):
        pc = per_core[c]
        perm, degp, s_sorted, starts = (
            pc["perm"], pc["degp"], pc["s_sorted"], pc["starts"],
        )
        node1 = np.zeros((ncols_pad, 128), dtype=np.int64)
        valid = np.zeros((ncols_pad, 128), dtype=bool)
        for t in range(NT):
            base = col0[t]
            d_orig = perm[t * 128 : (t + 1) * 128]
            real = d_orig >= 0
            dg = np.where(real, d_orig, 0)
            # column 0: the dst's own row (self loop + al_dst source)
            node1[base, :] = c * ND + dg
            valid[base, :] = real
            kt = int(K[t])
            if kt > 1:
                st = starts[dg]
                cnt = degp[t * 128 : (t + 1) * 128]
                for j in range(1, kt):
                    sel = (j - 1 < cnt) & real
                    idxs = st + (j - 1)
                    node1[base + j, sel] = s_sorted[np.where(sel, idxs, 0)][sel]
                    valid[base + j, sel] = True
        datas.append(dict(node1=node1, valid=valid, perm=perm))
    return datas, K, col0, nchunks, ncols_pad


_BUILD_CACHE = {}


def _build(K, col0, nchunks):
    import concourse.bass as bass
    import concourse.bacc as bacc
    import concourse.mybir as mybir
    import concourse.tile as tile
    from concourse.masks import make_identity

    f32 = mybir.dt.float32
    bf16 = mybir.dt.bfloat16
    i16 = mybir.dt.int16
    AX = mybir.AxisListType.X
    OP = mybir.AluOpType
    ACT = mybir.ActivationFunctionType

    ncols_pad = nchunks * CPC
    NI = CPC * 128           # indices per gather call
    IW = NI // 16            # wrapped idx cols per chunk

    # tile segments per gather chunk: (tile, gc0, gc1, first, last)
    segs_of = [[] for _ in range(nchunks)]
    for t in range(NT):
        a = int(col0[t])
        b = a + int(K[t])
        c = a
        while c < b:
            k = c // CPC
            c1 = min(b, (k + 1) * CPC)
            segs_of[k].append((t, c, c1, c == a, c1 == b))
            c = c1

    nc = bacc.Bacc("TRN2", target_bir_lowering=False, debug=False,
                   num_devices=NCORES, num_swdge_queues=4)

    xt = nc.dram_tensor("xt", [IN, NPAD], f32, kind="ExternalInput")
    w1e = nc.dram_tensor("w1e", [IN, 80], f32, kind="ExternalInput")
    b1e = nc.dram_tensor("b1e", [128, 80], f32, kind="ExternalInput")
    w2e = nc.dram_tensor("w2e", [HID, 4], bf16, kind="ExternalInput")
    b2e = nc.dram_tensor("b2e", [128, 4], bf16, kind="ExternalInput")
    pr1 = nc.dram_tensor("pr1", [88, 320], bf16, kind="ExternalInput")
    pr2 = nc.dram_tensor("pr2", [1, T2W], bf16, kind="ExternalInput")
    idx1d = nc.dram_tensor("idx1d", [128, nchunks * IW], i16,
                           kind="ExternalInput")
    idx2d = nc.dram_tensor("idx2d", [128, nchunks * IW], i16,
                           kind="ExternalInput")
    oh1d = nc.dram_tensor("oh1d", [128, ncols_pad, 4], bf16,
                          kind="ExternalInput")
    oh2d = nc.dram_tensor("oh2d", [128, ncols_pad, 4], bf16,
                          kind="ExternalInput")

    table1 = nc.dram_tensor("table1", [T1R, T1W], bf16, kind="Internal")
    t2shard = nc.dram_tensor("t2shard", [PT // 4, T2W], bf16, kind="Internal")
    table2 = nc.dram_tensor("table2", [T2R, T2W], bf16, kind="Internal",
                            addr_space="Shared")
    outp = nc.dram_tensor("outp", [PT, OUT], f32, kind="ExternalOutput")

    with tile.TileContext(nc) as tc:
        with (
            tc.tile_pool(name="const", bufs=1) as cpool,
            tc.tile_pool(name="node", bufs=3) as npool,
            tc.tile_pool(name="npsum", bufs=3, space="PSUM") as npsum,
            tc.tile_pool(name="gth", bufs=3) as gpool,
            tc.tile_pool(name="edge", bufs=3) as epool,
            tc.tile_pool(name="accs", bufs=3) as apool,
            tc.tile_pool(name="fin", bufs=2) as fpool,
            tc.tile_pool(name="fpsum", bufs=2, space="PSUM") as fpsum,
        ):
            ident = cpool.tile([128, 128], bf16)
            make_identity(nc, ident[:])
            w1es = cpool.tile([IN, 80], f32)
            nc.sync.dma_start(out=w1es[:], in_=w1e[:])
            b1es = cpool.tile([128, 80], f32)
            nc.sync.dma_start(out=b1es[:], in_=b1e[:])
            w2es = cpool.tile([HID, 4], bf16)
            nc.sync.dma_start(out=w2es[:], in_=w2e[:])
            b2es = cpool.tile([128, 4], bf16)
            nc.sync.dma_start(out=b2es[:], in_=b2e[:])
            pr1s = cpool.tile([88, 320], bf16)
            nc.sync.dma_start(out=pr1s[:], in_=pr1[:])
            pr2s = cpool.tile([1, T2W], bf16)
            nc.sync.dma_start(out=pr2s[:], in_=pr2[:])

            # ---- node phase: table1 = 4-pack [al_src | h+b1 | al_dst]
            # iteration i, partition p, sub j -> node i*512 + 4p + j
            # -> table1 row i*128 + p, elems j*80 .. j*80+80
            for i in range(NITER):
                xti = npool.tile([128, 512], f32, tag="xt")
                nc.scalar.dma_start(out=xti[:], in_=xt[:, i * 512:(i + 1) * 512])
                g1 = npsum.tile([128, 4, 80], f32, tag="g1")
                for j in range(4):
                    nc.tensor.matmul(out=g1[:, j, :],
                                     lhsT=xti[:, j * 128:(j + 1) * 128],
                                     rhs=w1es[:], start=True, stop=True)
                t1s = npool.tile([128, 4, 80], bf16, tag="t1")
                nc.vector.tensor_tensor(
                    out=t1s[:], in0=g1[:],
                    in1=b1es[:].unsqueeze(1).to_broadcast([128, 4, 80]),
                    op=OP.add)
                dst_ap = table1[i * 128:(i + 1) * 128, 0:320].rearrange(
                    "r (j v) -> r j v", v=80)
                nc.sync.dma_start(out=dst_ap, in_=t1s[:])
            # poison rows for pad slots: al_src/al_dst = -1e30, h = 0
            nc.sync.dma_start(
                out=table1[PR1:PR1 + 88, 0:320], in_=pr1s[:])

            def select4(out_ap, gt, j0, kt, voff, nv, oh_t, ew, tag):
                tmp = epool.tile([128, CPC, nv], bf16, tag=f"sel{tag}")
                nc.vector.tensor_tensor(
                    out=out_ap,
                    in0=gt[:, j0:j0 + kt, voff:voff + nv],
                    in1=oh_t[:, j0:j0 + kt, 0:1].to_broadcast([128, kt, nv]),
                    op=OP.mult)
                for i in range(1, 4):
                    nc.vector.tensor_tensor(
                        out=tmp[:, 0:kt, :],
                        in0=gt[:, j0:j0 + kt, i * ew + voff:i * ew + voff + nv],
                        in1=oh_t[:, j0:j0 + kt, i:i + 1].to_broadcast(
                            [128, kt, nv]),
                        op=OP.mult)
                    nc.vector.tensor_tensor(out=out_ap, in0=out_ap,
                                            in1=tmp[:, 0:kt, :], op=OP.add)

            # ---- edge phases
            gq = [0]

            def edge_phase(layer):
                if layer == 1:
                    idxd, ohd, tab, EW, EWN, NH, NCH = (
                        idx1d, oh1d, table1, T1W, 80, H1, C1)
                else:
                    idxd, ohd, tab, EW, EWN, NH, NCH = (
                        idx2d, oh2d, table2, T2W, 4, 1, OUT)
                NV = NH + NH * NCH  # 72 / 3
                WR = NH + NH * NCH
                state = {}

                for k in range(nchunks):
                    eng = nc.sync if k % 2 == 0 else nc.scalar
                    idx_t = epool.tile([128, IW], i16, tag=f"ix{layer}")
                    eng.dma_start(out=idx_t[:],
                                  in_=idxd[:, k * IW:(k + 1) * IW])
                    oh_t = epool.tile([128, CPC, 4], bf16, tag=f"oh{layer}")
                    eng.dma_start(out=oh_t[:],
                                  in_=ohd[:, k * CPC:(k + 1) * CPC, :])
                    gt = gpool.tile([128, CPC, EW], bf16, tag=f"gt{layer}")
                    nc.gpsimd.dma_gather(
                        gt[:], tab[:], idx_t[:], NI, NI, EW,
                        queue_num=gq[0] % 4)
                    gq[0] += 1
                    for (t, gc0, gc1, first, last) in segs_of[k]:
                        c0 = gc0 - k * CPC
                        kt = gc1 - gc0
                        V = epool.tile([128, CPC, NV], bf16, tag=f"V{layer}")
                        select4(V[:, 0:kt, :], gt, c0, kt, 0, NV, oh_t, EWN,
                                f"v{layer}")
                        if first:
                            adt = apool.tile([128, 1, NH], bf16,
                                             tag=f"adt{layer}")
                            select4(adt[:], gt, c0, 1, NV, NH, oh_t, EWN,
                                    f"a{layer}")
                            acc = apool.tile([128, WR], f32, tag=f"acc{layer}")
                            state[t] = (adt, acc)
                        else:
                            adt, acc = state[t]
                        eT = epool.tile([128, CPC, NH], f32, tag=f"e{layer}")
                        nc.vector.tensor_tensor(
                            out=eT[:, 0:kt, :], in0=V[:, 0:kt, 0:NH],
                            in1=adt[:].to_broadcast([128, kt, NH]),
                            op=OP.add)
                        lk = epool.tile([128, CPC, NH], f32, tag=f"lk{layer}")
                        nc.vector.tensor_scalar(out=lk[:, 0:kt, :],
                                                in0=eT[:, 0:kt, :],
                                                scalar1=NEG, scalar2=None,
                                                op0=OP.mult)
                        nc.vector.tensor_tensor(out=lk[:, 0:kt, :],
                                                in0=lk[:, 0:kt, :],
                                                in1=eT[:, 0:kt, :], op=OP.max)
                        W = epool.tile([128, WR, CPC], bf16, tag=f"W{layer}")
                        nc.scalar.activation(
                            out=W[:, 0:NH, 0:kt].rearrange("p h c -> p c h"),
                            in_=lk[:, 0:kt, :], func=ACT.Exp)
                        nc.vector.tensor_tensor(
                            out=W[:, NH:WR, 0:kt].rearrange(
                                "p (h c) j -> p h c j", h=NH),
                            in0=V[:, 0:kt, NH:NH + NH * NCH].rearrange(
                                "p j (h c) -> p h c j", h=NH),
                            in1=W[:, 0:NH, 0:kt].unsqueeze(2).to_broadcast(
                                [128, NH, NCH, kt]),
                            op=OP.mult)
                        if first:
                            nc.vector.tensor_reduce(
                                out=acc[:], in_=W[:, :, 0:kt], axis=AX,
                                op=OP.add)
                        else:
                            red = apool.tile([128, WR], f32, tag=f"red{layer}")
                            nc.vector.tensor_reduce(
                                out=red[:], in_=W[:, :, 0:kt], axis=AX,
                                op=OP.add)
                            nc.vector.tensor_tensor(out=acc[:], in0=acc[:],
                                                    in1=red[:], op=OP.add)
                        if last:
                            finalize(layer, t, acc)
                            del state[t]

            def finalize(layer, t, a):
                NH = H1 if layer == 1 else 1
                NCH = C1 if layer == 1 else OUT
                WR = NH + NH * NCH
                rden = fpool.tile([128, NH], f32, tag=f"rden{layer}")
                nc.vector.reciprocal(out=rden[:], in_=a[:, 0:NH])
                if layer == 1:
                    z = fpool.tile([128, HID], f32, tag="z")
                    nc.vector.tensor_tensor(
                        out=z[:].rearrange("p (h c) -> p h c", h=NH),
                        in0=a[:, NH:WR].rearrange("p (h c) -> p h c", h=NH),
                        in1=rden[:].unsqueeze(2).to_broadcast([128, NH, NCH]),
                        op=OP.mult)
                    # elu -> bf16
                    zm = fpool.tile([128, HID], f32, tag="zm")
                    nc.vector.tensor_scalar(out=zm[:], in0=z[:], scalar1=0.0,
                                            scalar2=None, op0=OP.min)
                    ze = fpool.tile([128, HID], f32, tag="ze")
                    nc.scalar.activation(out=ze[:], in_=zm[:], func=ACT.Exp)
                    nc.vector.tensor_scalar(out=ze[:], in0=ze[:], scalar1=-1.0,
                                            scalar2=None, op0=OP.add)
                    nc.vector.tensor_scalar(out=zm[:], in0=z[:], scalar1=0.0,
                                            scalar2=None, op0=OP.max)
                    zb = fpool.tile([128, HID], bf16, tag="zb")
                    nc.vector.tensor_tensor(out=zb[:], in0=zm[:], in1=ze[:],
                                            op=OP.add)
                    # table2 row = z @ W2e + b2e
                    zT_ps = fpsum.tile([HID, 128], bf16, tag="zTp")
                    nc.tensor.transpose(out=zT_ps[:], in_=zb[:],
                                        identity=ident[:])
                    zTs = fpool.tile([HID, 128], bf16, tag="zTs")
                    nc.vector.tensor_copy(out=zTs[:], in_=zT_ps[:])
                    g2 = fpsum.tile([128, 4], f32, tag="g2p")
                    nc.tensor.matmul(out=g2[:], lhsT=zTs[:], rhs=w2es[:],
                                     start=True, stop=True)
                    g2s = fpool.tile([128, 4], bf16, tag="g2s")
                    nc.vector.tensor_tensor(out=g2s[:], in0=g2[:], in1=b2es[:],
                                            op=OP.add)
                    dst_ap = t2shard[t * 32:t * 32 + 32, 0:16].rearrange(
                        "r (n v) -> r n v", v=4)
                    nc.sync.dma_start(out=dst_ap, in_=g2s[:])
                else:
                    o = fpool.tile([128, OUT], f32, tag="o2")
                    nc.vector.tensor_tensor(
                        out=o[:], in0=a[:, 1:1 + OUT],
                        in1=rden[:].to_broadcast([128, OUT]),
                        op=OP.mult)
                    nc.sync.dma_start(
                        out=outp[t * 128:(t + 1) * 128, :], in_=o[:])

            edge_phase(1)

            # ---- exchange layer-2 node features
            nc.gpsimd.collective_compute(
                "AllGather",
                mybir.AluOpType.bypass,
                replica_groups=[list(range(NCORES))],
                ins=[t2shard[:]],
                outs=[table2[:]],
            )
            # poison row for layer-2 pad slots
            nc.sync.dma_start(out=table2[PR2:PR2 + 1, :], in_=pr2s[:])

            edge_phase(2)

    nc.compile()
    return nc


def kernel(**inputs):
    from concourse.bass_utils import run_bass_kernel_spmd

    x = np.asarray(inputs["x"], dtype=np.float32)
    ei = np.asarray(inputs["edge_index"]).astype(np.int64)
    w1 = np.asarray(inputs["W1"], dtype=np.float32)
    a1s = np.asarray(inputs["a1_src"], dtype=np.float32)
    a1d = np.asarray(inputs["a1_dst"], dtype=np.float32)
    b1 = np.asarray(inputs["b1"], dtype=np.float32)
    w2 = np.asarray(inputs["W2"], dtype=np.float32)
    a2s = np.asarray(inputs["a2_src"], dtype=np.float32)
    a2d = np.asarray(inputs["a2_dst"], dtype=np.float32)
    b2 = np.asarray(inputs["b2"], dtype=np.float32)

    src = ei[0]
    dst = ei[1]

    datas, K, col0, nchunks, ncols_pad = _plan(src, dst)

    # permuted global position of each node for the L2 table
    gpos_of_node = np.zeros(NPAD, dtype=np.int64)
    for c in range(NCORES):
        perm = datas[c]["perm"]
        real = perm >= 0
        gpos_of_node[c * ND + perm[real]] = c * PT + np.nonzero(real)[0]

    # weights
    A1s = np.zeros((HID, H1), dtype=np.float32)
    A1d = np.zeros((HID, H1), dtype=np.float32)
    for h in range(H1):
        A1s[h * C1:(h + 1) * C1, h] = a1s[h]
        A1d[h * C1:(h + 1) * C1, h] = a1d[h]
    w1e = np.concatenate([w1 @ A1s, w1, w1 @ A1d], axis=1)      # [128, 80]
    w2e = np.concatenate([w2 @ a2s.T, w2, w2 @ a2d.T], axis=1)  # [64, 4]
    b1e = np.zeros((128, 80), dtype=np.float32)
    b1e[:, H1:H1 + HID] = b1[None, :]
    b2e = np.zeros((128, 4), dtype=BF16)
    b2e[:, 1:1 + OUT] = b2[None, :].astype(BF16)
    # poison rows: per sub-node [al_src(8)=BIG | h(64)=0 | al_dst(8)=BIG]
    sub = np.zeros(80, dtype=np.float32)
    sub[0:H1] = BIG
    sub[H1 + HID:] = BIG
    pr1 = np.tile(sub, (88, 4)).astype(BF16)                    # [88, 320]
    pr2 = np.zeros((1, T2W), dtype=BF16)
    sub2 = np.zeros(4, dtype=np.float32)
    sub2[0] = BIG
    sub2[3] = BIG
    pr2[0, 0:16] = np.tile(sub2, 4).astype(BF16)

    # x transposed + node-phase interleave: xt[c, i*512 + j*128 + p] =
    # x[i*512 + 4p + j, c]
    xp = np.zeros((NPAD, IN), dtype=np.float32)
    xp[:N] = x
    xr = xp.reshape(NITER, 128, 4, IN)            # [i, p, j, c]
    xt = np.ascontiguousarray(xr.transpose(3, 0, 2, 1).reshape(IN, NPAD))

    key = (nchunks, tuple(K.tolist()))
    if key not in _BUILD_CACHE:
        _BUILD_CACHE[key] = _build(K, col0, nchunks)
    nc = _BUILD_CACHE[key]

    common = dict(xt=xt, w1e=w1e, b1e=b1e, w2e=w2e.astype(BF16), b2e=b2e,
                  pr1=pr1, pr2=pr2)
    eye4 = np.eye(4, dtype=np.float32)
    in_maps = []
    for c in range(NCORES):
        node1 = datas[c]["node1"]           # [ncols_pad, 128]
        valid = datas[c]["valid"]
        g = gpos_of_node[node1]

        idx1 = np.where(valid, node1 // 4, PR1).astype(np.int16)
        idx2 = np.where(valid, g // 4, PR2).astype(np.int16)
        oh1 = eye4[np.where(valid, node1 % 4, 0)].astype(BF16)  # [nc,128,4]
        oh2 = eye4[np.where(valid, g % 4, 0)].astype(BF16)

        idx1w = np.concatenate(
            [_wrap_idx(idx1[k * CPC:(k + 1) * CPC].reshape(-1))
             for k in range(nchunks)], axis=1)
        idx2w = np.concatenate(
            [_wrap_idx(idx2[k * CPC:(k + 1) * CPC].reshape(-1))
             for k in range(nchunks)], axis=1)

        m = dict(common)
        m["idx1d"] = idx1w
        m["idx2d"] = idx2w
        m["oh1d"] = np.ascontiguousarray(oh1.transpose(1, 0, 2))
        m["oh2d"] = np.ascontiguousarray(oh2.transpose(1, 0, 2))
        in_maps.append(m)

    global _LAST_IN_MAPS
    _LAST_IN_MAPS = in_maps
    res = run_bass_kernel_spmd(nc, in_maps, list(range(NCORES)))

    out = np.zeros((N, OUT), dtype=np.float32)
    for c in range(NCORES):
        op = res.results[c]["outp"]       # [PT, 2] in permuted order
        perm = datas[c]["perm"]
        real = perm >= 0
        out[c * ND + perm[real]] = op[real]
    return out


# revision 7
# speedup vs baseline: 1.7274x; 1.7274x over previous
"""GAT (2-layer, PyG-style) on 8 Trainium2 NeuronCores via Bass/Tile.

Strategy (dst-sharded, degree-sorted tiles, big-batch dma_gather):
- Nodes sharded by dst across 8 cores (12500 each). Per core, dsts are
  degree-sorted and grouped into 98 tiles of 128 (partition = dst).
- Per tile, column 0 gathers the dst's own table row (serving both the
  self-loop edge and the per-partition al_dst values); remaining columns
  hold in-edges, padded to the tile max degree.
- Layer tables are 4-node-packed rows (<=25088 rows, int16 dma_gather
  indices); a 4-way one-hot select on DVE picks the node within the row.
- Gathers run 4096 indices per call round-robin over the 4 SWDGE queues
  (32 columns per chunk; chunks span tile boundaries).
- Pad slots index dedicated poison rows (al = -1e30) so exp() kills
  them; no mask arrays needed.
- Segment softmax: no max-subtraction needed (logits are O(1)); the
  denominator divides the aggregated numerator once per dst row.
- b1/b2 folded into the h-columns of the tables (alpha sums to 1).
- x is pre-transposed on the host so the node phase is matmul-only
  (no PE transposes); layer-1 table writes are 640 B contiguous.
- Layer-2 per-node features (4 values) travel via AllGather of the
  per-core shards in core-local permuted order.
"""

import numpy as np
import ml_dtypes

BF16 = ml_dtypes.bfloat16

N = 100_000
E = 3_200_000
IN = 128
H1, C1 = 8, 8
HID = H1 * C1          # 64
OUT = 2
NEG = 0.2
NCORES = 8
ND = N // NCORES       # dsts per core: 12500
NT = 98                # tiles per core (98*128 = 12544)
PT = NT * 128          # padded dst slots per core: 12544
NPAD = 100_352         # x padded to 784*128
NITER = NPAD // 512    # node-phase iterations (512 nodes each): 196
CPC = 32               # compute-chunk columns (4 gather calls each)
GPC = 8                # columns per dma_gather call (1024 indices)
T1R = NPAD // 4        # 25088 4-pack rows for layer-1 table
T1W = 384              # bf16 elems per table1 row (768 B); 4 x 80 used
T2R = (PT * NCORES) // 4   # 25088
T2W = 128              # bf16 elems per table2 row (256 B); 4 x 4 used
PR1 = NPAD // 4 - 88   # first poison 4-pack row (nodes 100000..100351)
PR2 = T2R - 1          # poison row in table2 (always unused slots)
BIG = -1e30


def _wrap_idx(flat):
    """int16 index array -> [128, n/16] wrapped-in-16-partitions, x8."""
    n = flat.shape[0]
    assert n % 16 == 0
    w = flat.reshape(n // 16, 16).T            # [16, n/16]
    return np.tile(w, (8, 1)).astype(np.int16)  # [128, n/16]


def _plan(src, dst):
    """Host-side index planning. Returns per-core slot arrays + schedule."""
    core = dst // ND
    dloc = dst % ND

    per_core = []
    for c in range(NCORES):
        m = core == c
        s_c = src[m]
        d_c = dloc[m]
        deg = np.bincount(d_c, minlength=ND)  # in-edges, no self loop yet
        order = np.argsort(-deg, kind="stable")  # degree-desc permutation
        perm = np.full(PT, -1, dtype=np.int64)
        perm[:ND] = order
        degp = np.zeros(PT, dtype=np.int64)
        degp[:ND] = deg[order]
        sort_by_d = np.argsort(d_c, kind="stable")
        s_sorted = s_c[sort_by_d]
        starts = np.zeros(ND + 1, dtype=np.int64)
        np.cumsum(deg, out=starts[1:])
        per_core.append(dict(perm=perm, degp=degp, s_sorted=s_sorted, starts=starts))

    # common K_t schedule: columns per tile = 1 (self/dst col) + max in-degree
    K = np.zeros(NT, dtype=np.int64)
    for t in range(NT):
        mx = 0
        for c in range(NCORES):
            d = per_core[c]["degp"][t * 128 : (t + 1) * 128]
            mx = max(mx, int(d.max()) if d.size else 0)
        K[t] = mx + 1
    ncols = int(K.sum())
    nchunks = (ncols + CPC - 1) // CPC
    ncols_pad = nchunks * CPC

    col0 = np.zeros(NT, dtype=np.int64)
    pos = 0
    for t in range(NT):
        col0[t] = pos
        pos += K[t]

    datas = []
    for c in range(NCORES):
        pc = per_core[c]
        perm, degp, s_sorted, starts = (
            pc["perm"], pc["degp"], pc["s_sorted"], pc["starts"],
        )
        node1 = np.zeros((ncols_pad, 128), dtype=np.int64)
        valid = np.zeros((ncols_pad, 128), dtype=bool)
        for t in range(NT):
            base = col0[t]
            d_orig = perm[t * 128 : (t + 1) * 128]
            real = d_orig >= 0
            dg = np.where(real, d_orig, 0)
            # column 0: the dst's own row (self loop + al_dst source)
            node1[base, :] = c * ND + dg
            valid[base, :] = real
            kt = int(K[t])
            if kt > 1:
                st = starts[dg]
                cnt = degp[t * 128 : (t + 1) * 128]
                for j in range(1, kt):
                    sel = (j - 1 < cnt) & real
                    idxs = st + (j - 1)
                    node1[base + j, sel] = s_sorted[np.where(sel, idxs, 0)][sel]
                    valid[base + j, sel] = True
        datas.append(dict(node1=node1, valid=valid, perm=perm))
    return datas, K, col0, nchunks, ncols_pad


_BUILD_CACHE = {}


def _build(K, col0, nchunks):
    import concourse.bass as bass
    import concourse.bacc as bacc
    import concourse.mybir as mybir
    import concourse.tile as tile
    from concourse.masks import make_identity

    f32 = mybir.dt.float32
    bf16 = mybir.dt.bfloat16
    i16 = mybir.dt.int16
    AX = mybir.AxisListType.X
    OP = mybir.AluOpType
    ACT = mybir.ActivationFunctionType

    ncols_pad = nchunks * CPC
    NI = GPC * 128           # indices per gather call: 1024
    IW = NI // 16            # wrapped idx cols per call: 64
    NCALL = CPC // GPC       # gather calls per chunk: 4

    # tile segments per gather chunk: (tile, gc0, gc1, first, last)
    segs_of = [[] for _ in range(nchunks)]
    for t in range(NT):
        a = int(col0[t])
        b = a + int(K[t])
        c = a
        while c < b:
            k = c // CPC
            c1 = min(b, (k + 1) * CPC)
            segs_of[k].append((t, c, c1, c == a, c1 == b))
            c = c1

    nc = bacc.Bacc("TRN2", target_bir_lowering=False, debug=False,
                   num_devices=NCORES, num_swdge_queues=4)

    xt = nc.dram_tensor("xt", [IN, NPAD], f32, kind="ExternalInput")
    w1e = nc.dram_tensor("w1e", [IN, 80], f32, kind="ExternalInput")
    b1e = nc.dram_tensor("b1e", [128, 80], f32, kind="ExternalInput")
    w2e = nc.dram_tensor("w2e", [HID, 4], bf16, kind="ExternalInput")
    b2e = nc.dram_tensor("b2e", [128, 4], bf16, kind="ExternalInput")
    pr1 = nc.dram_tensor("pr1", [88, 320], bf16, kind="ExternalInput")
    pr2 = nc.dram_tensor("pr2", [1, T2W], bf16, kind="ExternalInput")
    idx1d = nc.dram_tensor("idx1d", [128, nchunks * NCALL * IW], i16,
                           kind="ExternalInput")
    idx2d = nc.dram_tensor("idx2d", [128, nchunks * NCALL * IW], i16,
                           kind="ExternalInput")
    oh1d = nc.dram_tensor("oh1d", [128, ncols_pad, 4], bf16,
                          kind="ExternalInput")
    oh2d = nc.dram_tensor("oh2d", [128, ncols_pad, 4], bf16,
                          kind="ExternalInput")

    table1 = nc.dram_tensor("table1", [T1R, T1W], bf16, kind="Internal")
    t2shard = nc.dram_tensor("t2shard", [PT // 4, T2W], bf16, kind="Internal")
    table2 = nc.dram_tensor("table2", [T2R, T2W], bf16, kind="Internal",
                            addr_space="Shared")
    outp = nc.dram_tensor("outp", [PT, OUT], f32, kind="ExternalOutput")

    with tile.TileContext(nc) as tc:
        with (
            tc.tile_pool(name="const", bufs=1) as cpool,
            tc.tile_pool(name="node", bufs=3) as npool,
            tc.tile_pool(name="npsum", bufs=3, space="PSUM") as npsum,
            tc.tile_pool(name="gth", bufs=3) as gpool,
            tc.tile_pool(name="edge", bufs=3) as epool,
            tc.tile_pool(name="accs", bufs=3) as apool,
            tc.tile_pool(name="fin", bufs=2) as fpool,
            tc.tile_pool(name="fpsum", bufs=2, space="PSUM") as fpsum,
        ):
            ident = cpool.tile([128, 128], bf16)
            make_identity(nc, ident[:])
            w1es = cpool.tile([IN, 80], f32)
            nc.sync.dma_start(out=w1es[:], in_=w1e[:])
            b1es = cpool.tile([128, 80], f32)
            nc.sync.dma_start(out=b1es[:], in_=b1e[:])
            w2es = cpool.tile([HID, 4], bf16)
            nc.sync.dma_start(out=w2es[:], in_=w2e[:])
            b2es = cpool.tile([128, 4], bf16)
            nc.sync.dma_start(out=b2es[:], in_=b2e[:])
            pr1s = cpool.tile([88, 320], bf16)
            nc.sync.dma_start(out=pr1s[:], in_=pr1[:])
            pr2s = cpool.tile([1, T2W], bf16)
            nc.sync.dma_start(out=pr2s[:], in_=pr2[:])

            # ---- node phase: table1 = 4-pack [al_src | h+b1 | al_dst]
            # iteration i, partition p, sub j -> node i*512 + 4p + j
            # -> table1 row i*128 + p, elems j*80 .. j*80+80
            for i in range(NITER):
                xti = npool.tile([128, 512], f32, tag="xt")
                nc.scalar.dma_start(out=xti[:], in_=xt[:, i * 512:(i + 1) * 512])
                g1 = npsum.tile([128, 4, 80], f32, tag="g1")
                for j in range(4):
                    nc.tensor.matmul(out=g1[:, j, :],
                                     lhsT=xti[:, j * 128:(j + 1) * 128],
                                     rhs=w1es[:], start=True, stop=True)
                t1s = npool.tile([128, 4, 80], bf16, tag="t1")
                nc.vector.tensor_tensor(
                    out=t1s[:], in0=g1[:],
                    in1=b1es[:].unsqueeze(1).to_broadcast([128, 4, 80]),
                    op=OP.add)
                dst_ap = table1[i * 128:(i + 1) * 128, 0:320].rearrange(
                    "r (j v) -> r j v", v=80)
                nc.sync.dma_start(out=dst_ap, in_=t1s[:])
            # poison rows for pad slots: al_src/al_dst = -1e30, h = 0
            nc.sync.dma_start(
                out=table1[PR1:PR1 + 88, 0:320], in_=pr1s[:])

            def select4(out_ap, gt, j0, kt, voff, nv, oh_t, ew, tag):
                tmp = epool.tile([128, CPC, nv], bf16, tag=f"sel{tag}")
                nc.vector.tensor_tensor(
                    out=out_ap,
                    in0=gt[:, j0:j0 + kt, voff:voff + nv],
                    in1=oh_t[:, j0:j0 + kt, 0:1].to_broadcast([128, kt, nv]),
                    op=OP.mult)
                for i in range(1, 4):
                    nc.vector.tensor_tensor(
                        out=tmp[:, 0:kt, :],
                        in0=gt[:, j0:j0 + kt, i * ew + voff:i * ew + voff + nv],
                        in1=oh_t[:, j0:j0 + kt, i:i + 1].to_broadcast(
                            [128, kt, nv]),
                        op=OP.mult)
                    nc.vector.tensor_tensor(out=out_ap, in0=out_ap,
                                            in1=tmp[:, 0:kt, :], op=OP.add)

            # ---- edge phases
            gq = [0]

            def edge_phase(layer):
                if layer == 1:
                    idxd, ohd, tab, EW, EWN, NH, NCH = (
                        idx1d, oh1d, table1, T1W, 80, H1, C1)
                else:
                    idxd, ohd, tab, EW, EWN, NH, NCH = (
                        idx2d, oh2d, table2, T2W, 4, 1, OUT)
                NV = NH + NH * NCH  # 72 / 3
                WR = NH + NH * NCH
                state = {}

                for k in range(nchunks):
                    eng = nc.sync if k % 2 == 0 else nc.scalar
                    idx_t = epool.tile([128, NCALL * IW], i16, tag=f"ix{layer}")
                    eng.dma_start(
                        out=idx_t[:],
                        in_=idxd[:, k * NCALL * IW:(k + 1) * NCALL * IW])
                    oh_t = epool.tile([128, CPC, 4], bf16, tag=f"oh{layer}")
                    eng.dma_start(out=oh_t[:],
                                  in_=ohd[:, k * CPC:(k + 1) * CPC, :])
                    gt = gpool.tile([128, CPC, EW], bf16, tag=f"gt{layer}")
                    for q in range(NCALL):
                        nc.gpsimd.dma_gather(
                            gt[:, q * GPC:(q + 1) * GPC, :], tab[:],
                            idx_t[:, q * IW:(q + 1) * IW], NI, NI, EW,
                            queue_num=gq[0] % 4)
                        gq[0] += 1
                    for (t, gc0, gc1, first, last) in segs_of[k]:
                        c0 = gc0 - k * CPC
                        kt = gc1 - gc0
                        V = epool.tile([128, CPC, NV], bf16, tag=f"V{layer}")
                        select4(V[:, 0:kt, :], gt, c0, kt, 0, NV, oh_t, EWN,
                                f"v{layer}")
                        if first:
                            adt = apool.tile([128, 1, NH], bf16,
                                             tag=f"adt{layer}")
                            select4(adt[:], gt, c0, 1, NV, NH, oh_t, EWN,
                                    f"a{layer}")
                            acc = apool.tile([128, WR], f32, tag=f"acc{layer}")
                            state[t] = (adt, acc)
                        else:
                            adt, acc = state[t]
                        eT = epool.tile([128, CPC, NH], f32, tag=f"e{layer}")
                        nc.vector.tensor_tensor(
                            out=eT[:, 0:kt, :], in0=V[:, 0:kt, 0:NH],
                            in1=adt[:].to_broadcast([128, kt, NH]),
                            op=OP.add)
                        lk = epool.tile([128, CPC, NH], f32, tag=f"lk{layer}")
                        nc.vector.tensor_scalar(out=lk[:, 0:kt, :],
                                                in0=eT[:, 0:kt, :],
                                                scalar1=NEG, scalar2=None,
                                                op0=OP.mult)
                        nc.vector.tensor_tensor(out=lk[:, 0:kt, :],
                                                in0=lk[:, 0:kt, :],
                                                in1=eT[:, 0:kt, :], op=OP.max)
                        W = epool.tile([128, WR, CPC], bf16, tag=f"W{layer}")
                        nc.scalar.activation(
                            out=W[:, 0:NH, 0:kt].rearrange("p h c -> p c h"),
                            in_=lk[:, 0:kt, :], func=ACT.Exp)
                        nc.vector.tensor_tensor(
                            out=W[:, NH:WR, 0:kt].rearrange(
                                "p (h c) j -> p h c j", h=NH),
                            in0=V[:, 0:kt, NH:NH + NH * NCH].rearrange(
                                "p j (h c) -> p h c j", h=NH),
                            in1=W[:, 0:NH, 0:kt].unsqueeze(2).to_broadcast(
                                [128, NH, NCH, kt]),
                            op=OP.mult)
                        if first:
                            nc.vector.tensor_reduce(
                                out=acc[:], in_=W[:, :, 0:kt], axis=AX,
                                op=OP.add)
                        else:
                            red = apool.tile([128, WR], f32, tag=f"red{layer}")
                            nc.vector.tensor_reduce(
                                out=red[:], in_=W[:, :, 0:kt], axis=AX,
                                op=OP.add)
                            nc.vector.tensor_tensor(out=acc[:], in0=acc[:],
                                                    in1=red[:], op=OP.add)
                        if last:
                            finalize(layer, t, acc)
                            del state[t]

            def finalize(layer, t, a):
                NH = H1 if layer == 1 else 1
                NCH = C1 if layer == 1 else OUT
                WR = NH + NH * NCH
                rden = fpool.tile([128, NH], f32, tag=f"rden{layer}")
                nc.vector.reciprocal(out=rden[:], in_=a[:, 0:NH])
                if layer == 1:
                    z = fpool.tile([128, HID], f32, tag="z")
                    nc.vector.tensor_tensor(
                        out=z[:].rearrange("p (h c) -> p h c", h=NH),
                        in0=a[:, NH:WR].rearrange("p (h c) -> p h c", h=NH),
                        in1=rden[:].unsqueeze(2).to_broadcast([128, NH, NCH]),
                        op=OP.mult)
                    # elu -> bf16
                    zm = fpool.tile([128, HID], f32, tag="zm")
                    nc.vector.tensor_scalar(out=zm[:], in0=z[:], scalar1=0.0,
                                            scalar2=None, op0=OP.min)
                    ze = fpool.tile([128, HID], f32, tag="ze")
                    nc.scalar.activation(out=ze[:], in_=zm[:], func=ACT.Exp)
                    nc.vector.tensor_scalar(out=ze[:], in0=ze[:], scalar1=-1.0,
                                            scalar2=None, op0=OP.add)
                    nc.vector.tensor_scalar(out=zm[:], in0=z[:], scalar1=0.0,
                                            scalar2=None, op0=OP.max)
                    zb = fpool.tile([128, HID], bf16, tag="zb")
                    nc.vector.tensor_tensor(out=zb[:], in0=zm[:], in1=ze[:],
                                            op=OP.add)
                    # table2 row = z @ W2e + b2e
                    zT_ps = fpsum.tile([HID, 128], bf16, tag="zTp")
                    nc.tensor.transpose(out=zT_ps[:], in_=zb[:],
                                        identity=ident[:])
                    zTs = fpool.tile([HID, 128], bf16, tag="zTs")
                    nc.vector.tensor_copy(out=zTs[:], in_=zT_ps[:])
                    g2 = fpsum.tile([128, 4], f32, tag="g2p")
                    nc.tensor.matmul(out=g2[:], lhsT=zTs[:], rhs=w2es[:],
                                     start=True, stop=True)
                    g2s = fpool.tile([128, 4], bf16, tag="g2s")
                    nc.vector.tensor_tensor(out=g2s[:], in0=g2[:], in1=b2es[:],
                                            op=OP.add)
                    dst_ap = t2shard[t * 32:t * 32 + 32, 0:16].rearrange(
                        "r (n v) -> r n v", v=4)
                    nc.sync.dma_start(out=dst_ap, in_=g2s[:])
                else:
                    o = fpool.tile([128, OUT], f32, tag="o2")
                    nc.vector.tensor_tensor(
                        out=o[:], in0=a[:, 1:1 + OUT],
                        in1=rden[:].to_broadcast([128, OUT]),
                        op=OP.mult)
                    nc.sync.dma_start(
                        out=outp[t * 128:(t + 1) * 128, :], in_=o[:])

            edge_phase(1)

            # ---- exchange layer-2 node features
            nc.gpsimd.collective_compute(
                "AllGather",
                mybir.AluOpType.bypass,
                replica_groups=[list(range(NCORES))],
                ins=[t2shard[:]],
                outs=[table2[:]],
            )
            # poison row for layer-2 pad slots
            nc.sync.dma_start(out=table2[PR2:PR2 + 1, :], in_=pr2s[:])

            edge_phase(2)

    nc.compile()
    return nc


def kernel(**inputs):
    from concourse.bass_utils import run_bass_kernel_spmd

    x = np.asarray(inputs["x"], dtype=np.float32)
    ei = np.asarray(inputs["edge_index"]).astype(np.int64)
    w1 = np.asarray(inputs["W1"], dtype=np.float32)
    a1s = np.asarray(inputs["a1_src"], dtype=np.float32)
    a1d = np.asarray(inputs["a1_dst"], dtype=np.float32)
    b1 = np.asarray(inputs["b1"], dtype=np.float32)
    w2 = np.asarray(inputs["W2"], dtype=np.float32)
    a2s = np.asarray(inputs["a2_src"], dtype=np.float32)
    a2d = np.asarray(inputs["a2_dst"], dtype=np.float32)
    b2 = np.asarray(inputs["b2"], dtype=np.float32)

    src = ei[0]
    dst = ei[1]

    datas, K, col0, nchunks, ncols_pad = _plan(src, dst)

    # permuted global position of each node for the L2 table
    gpos_of_node = np.zeros(NPAD, dtype=np.int64)
    for c in range(NCORES):
        perm = datas[c]["perm"]
        real = perm >= 0
        gpos_of_node[c * ND + perm[real]] = c * PT + np.nonzero(real)[0]

    # weights
    A1s = np.zeros((HID, H1), dtype=np.float32)
    A1d = np.zeros((HID, H1), dtype=np.float32)
    for h in range(H1):
        A1s[h * C1:(h + 1) * C1, h] = a1s[h]
        A1d[h * C1:(h + 1) * C1, h] = a1d[h]
    w1e = np.concatenate([w1 @ A1s, w1, w1 @ A1d], axis=1)      # [128, 80]
    w2e = np.concatenate([w2 @ a2s.T, w2, w2 @ a2d.T], axis=1)  # [64, 4]
    b1e = np.zeros((128, 80), dtype=np.float32)
    b1e[:, H1:H1 + HID] = b1[None, :]
    b2e = np.zeros((128, 4), dtype=BF16)
    b2e[:, 1:1 + OUT] = b2[None, :].astype(BF16)
    # poison rows: per sub-node [al_src(8)=BIG | h(64)=0 | al_dst(8)=BIG]
    sub = np.zeros(80, dtype=np.float32)
    sub[0:H1] = BIG
    sub[H1 + HID:] = BIG
    pr1 = np.tile(sub, (88, 4)).astype(BF16)                    # [88, 320]
    pr2 = np.zeros((1, T2W), dtype=BF16)
    sub2 = np.zeros(4, dtype=np.float32)
    sub2[0] = BIG
    sub2[3] = BIG
    pr2[0, 0:16] = np.tile(sub2, 4).astype(BF16)

    # x transposed + node-phase interleave: xt[c, i*512 + j*128 + p] =
    # x[i*512 + 4p + j, c]
    xp = np.zeros((NPAD, IN), dtype=np.float32)
    xp[:N] = x
    xr = xp.reshape(NITER, 128, 4, IN)            # [i, p, j, c]
    xt = np.ascontiguousarray(xr.transpose(3, 0, 2, 1).reshape(IN, NPAD))

    key = (nchunks, tuple(K.tolist()))
    if key not in _BUILD_CACHE:
        _BUILD_CACHE[key] = _build(K, col0, nchunks)
    nc = _BUILD_CACHE[key]

    common = dict(xt=xt, w1e=w1e, b1e=b1e, w2e=w2e.astype(BF16), b2e=b2e,
                  pr1=pr1, pr2=pr2)
    eye4 = np.eye(4, dtype=np.float32)
    in_maps = []
    for c in range(NCORES):
        node1 = datas[c]["node1"]           # [ncols_pad, 128]
        valid = datas[c]["valid"]
        g = gpos_of_node[node1]

        idx1 = np.where(valid, node1 // 4, PR1).astype(np.int16)
        idx2 = np.where(valid, g // 4, PR2).astype(np.int16)
        oh1 = eye4[np.where(valid, node1 % 4, 0)].astype(BF16)  # [nc,128,4]
        oh2 = eye4[np.where(valid, g % 4, 0)].astype(BF16)

        ngrp = ncols_pad // GPC
        idx1w = np.concatenate(
            [_wrap_idx(idx1[g * GPC:(g + 1) * GPC].reshape(-1))
             for g in range(ngrp)], axis=1)
        idx2w = np.concatenate(
            [_wrap_idx(idx2[g * GPC:(g + 1) * GPC].reshape(-1))
             for g in range(ngrp)], axis=1)

        m = dict(common)
        m["idx1d"] = idx1w
        m["idx2d"] = idx2w
        m["oh1d"] = np.ascontiguousarray(oh1.transpose(1, 0, 2))
        m["oh2d"] = np.ascontiguousarray(oh2.transpose(1, 0, 2))
        in_maps.append(m)

    global _LAST_IN_MAPS
    _LAST_IN_MAPS = in_maps
    res = run_bass_kernel_spmd(nc, in_maps, list(range(NCORES)))

    out = np.zeros((N, OUT), dtype=np.float32)
    for c in range(NCORES):
        op = res.results[c]["outp"]       # [PT, 2] in permuted order
        perm = datas[c]["perm"]
        real = perm >= 0
        out[c * ND + perm[real]] = op[real]
    return out


# revision 9
# speedup vs baseline: 2.0834x; 1.2061x over previous
"""GAT (2-layer, PyG-style) on 8 Trainium2 NeuronCores via Bass/Tile.

Strategy (dst-sharded, degree-sorted tiles, big-batch dma_gather):
- Nodes sharded by dst across 8 cores (12500 each). Per core, dsts are
  degree-sorted and grouped into 98 tiles of 128 (partition = dst).
- Per tile, column 0 gathers the dst's own table row (serving both the
  self-loop edge and the per-partition al_dst values); remaining columns
  hold in-edges, padded to the tile max degree.
- Layer tables are 4-node-packed rows (<=25088 rows, int16 dma_gather
  indices); a 4-way one-hot select on DVE picks the node within the row.
- Gathers run 4096 indices per call round-robin over the 4 SWDGE queues
  (32 columns per chunk; chunks span tile boundaries).
- Pad slots index dedicated poison rows (al = -1e30) so exp() kills
  them; no mask arrays needed.
- Segment softmax: no max-subtraction needed (logits are O(1)); the
  denominator divides the aggregated numerator once per dst row.
- b1/b2 folded into the h-columns of the tables (alpha sums to 1).
- x is pre-transposed on the host so the node phase is matmul-only
  (no PE transposes); layer-1 table writes are 640 B contiguous.
- Layer-2 per-node features (4 values) travel via AllGather of the
  per-core shards in core-local permuted order.
"""

import numpy as np
import ml_dtypes

BF16 = ml_dtypes.bfloat16

N = 100_000
E = 3_200_000
IN = 128
H1, C1 = 8, 8
HID = H1 * C1          # 64
OUT = 2
NEG = 0.2
NCORES = 8
ND = N // NCORES       # dsts per core: 12500
NT = 98                # tiles per core (98*128 = 12544)
PT = NT * 128          # padded dst slots per core: 12544
NPAD = 100_352         # x padded to 784*128
NITER = NPAD // 512    # node-phase iterations (512 nodes each): 196
CPC = 32               # compute-chunk columns (4 gather calls each)
GPC = 8                # columns per dma_gather call (1024 indices)
T1R = NPAD // 4        # 25088 4-pack rows for layer-1 table
T1W = 384              # bf16 elems per table1 row (768 B); 4 x 80 used
T2R = (PT * NCORES) // 4   # 25088
T2W = 128              # bf16 elems per table2 row (256 B); 4 x 4 used
PR1 = NPAD // 4 - 88   # first poison 4-pack row (nodes 100000..100351)
PR2 = T2R - 1          # poison row in table2 (always unused slots)
BIG = -1e30


def _wrap_idx(flat):
    """int16 index array -> [128, n/16] wrapped-in-16-partitions, x8."""
    n = flat.shape[0]
    assert n % 16 == 0
    w = flat.reshape(n // 16, 16).T            # [16, n/16]
    return np.tile(w, (8, 1)).astype(np.int16)  # [128, n/16]


def _plan(src, dst):
    """Host-side index planning. Returns per-core slot arrays + schedule."""
    core = dst // ND
    dloc = dst % ND

    per_core = []
    for c in range(NCORES):
        m = core == c
        s_c = src[m]
        d_c = dloc[m]
        deg = np.bincount(d_c, minlength=ND)  # in-edges, no self loop yet
        order = np.argsort(-deg, kind="stable")  # degree-desc permutation
        perm = np.full(PT, -1, dtype=np.int64)
        perm[:ND] = order
        degp = np.zeros(PT, dtype=np.int64)
        degp[:ND] = deg[order]
        sort_by_d = np.argsort(d_c, kind="stable")
        s_sorted = s_c[sort_by_d]
        starts = np.zeros(ND + 1, dtype=np.int64)
        np.cumsum(deg, out=starts[1:])
        per_core.append(dict(perm=perm, degp=degp, s_sorted=s_sorted, starts=starts))

    # common K_t schedule: columns per tile = 1 (self/dst col) + max in-degree
    K = np.zeros(NT, dtype=np.int64)
    for t in range(NT):
        mx = 0
        for c in range(NCORES):
            d = per_core[c]["degp"][t * 128 : (t + 1) * 128]
            mx = max(mx, int(d.max()) if d.size else 0)
        K[t] = mx + 1
    ncols = int(K.sum())
    nchunks = (ncols + CPC - 1) // CPC
    ncols_pad = nchunks * CPC

    col0 = np.zeros(NT, dtype=np.int64)
    pos = 0
    for t in range(NT):
        col0[t] = pos
        pos += K[t]

    datas = []
    for c in range(NCORES):
        pc = per_core[c]
        perm, degp, s_sorted, starts = (
            pc["perm"], pc["degp"], pc["s_sorted"], pc["starts"],
        )
        node1 = np.zeros((ncols_pad, 128), dtype=np.int64)
        valid = np.zeros((ncols_pad, 128), dtype=bool)
        for t in range(NT):
            base = col0[t]
            d_orig = perm[t * 128 : (t + 1) * 128]
            real = d_orig >= 0
            dg = np.where(real, d_orig, 0)
            # column 0: the dst's own row (self loop + al_dst source)
            node1[base, :] = c * ND + dg
            valid[base, :] = real
            kt = int(K[t])
            if kt > 1:
                st = starts[dg]
                cnt = degp[t * 128 : (t + 1) * 128]
                for j in range(1, kt):
                    sel = (j - 1 < cnt) & real
                    idxs = st + (j - 1)
                    node1[base + j, sel] = s_sorted[np.where(sel, idxs, 0)][sel]
                    valid[base + j, sel] = True
        datas.append(dict(node1=node1, valid=valid, perm=perm))
    return datas, K, col0, nchunks, ncols_pad


_BUILD_CACHE = {}


def _build(K, col0, nchunks):
    import concourse.bass as bass
    import concourse.bacc as bacc
    import concourse.mybir as mybir
    import concourse.tile as tile
    from concourse.masks import make_identity

    f32 = mybir.dt.float32
    bf16 = mybir.dt.bfloat16
    i16 = mybir.dt.int16
    AX = mybir.AxisListType.X
    OP = mybir.AluOpType
    ACT = mybir.ActivationFunctionType

    ncols_pad = nchunks * CPC
    NI = GPC * 128           # indices per gather call: 1024
    IW = NI // 16            # wrapped idx cols per call: 64
    NCALL = CPC // GPC       # gather calls per chunk: 4

    # tile segments per gather chunk: (tile, gc0, gc1, first, last)
    segs_of = [[] for _ in range(nchunks)]
    for t in range(NT):
        a = int(col0[t])
        b = a + int(K[t])
        c = a
        while c < b:
            k = c // CPC
            c1 = min(b, (k + 1) * CPC)
            segs_of[k].append((t, c, c1, c == a, c1 == b))
            c = c1

    nc = bacc.Bacc("TRN2", target_bir_lowering=False, debug=False,
                   num_devices=NCORES, num_swdge_queues=4)

    xt = nc.dram_tensor("xt", [IN, NPAD], f32, kind="ExternalInput")
    w1e = nc.dram_tensor("w1e", [IN, 80], f32, kind="ExternalInput")
    b1e = nc.dram_tensor("b1e", [128, 80], f32, kind="ExternalInput")
    w2e = nc.dram_tensor("w2e", [HID, 4], bf16, kind="ExternalInput")
    b2e = nc.dram_tensor("b2e", [128, 4], bf16, kind="ExternalInput")
    pr1 = nc.dram_tensor("pr1", [88, 320], bf16, kind="ExternalInput")
    pr2 = nc.dram_tensor("pr2", [1, T2W], bf16, kind="ExternalInput")
    idx1d = nc.dram_tensor("idx1d", [128, nchunks * NCALL * IW], i16,
                           kind="ExternalInput")
    idx2d = nc.dram_tensor("idx2d", [128, nchunks * NCALL * IW], i16,
                           kind="ExternalInput")
    u8 = mybir.dt.uint8
    oh1d = nc.dram_tensor("oh1d", [128, ncols_pad, 3], u8,
                          kind="ExternalInput")
    oh2d = nc.dram_tensor("oh2d", [128, ncols_pad, 3], u8,
                          kind="ExternalInput")

    table1 = nc.dram_tensor("table1", [T1R, T1W], bf16, kind="Internal")
    t2shard = nc.dram_tensor("t2shard", [PT // 4, T2W], bf16, kind="Internal")
    table2 = nc.dram_tensor("table2", [T2R, T2W], bf16, kind="Internal",
                            addr_space="Shared")
    outp = nc.dram_tensor("outp", [PT, OUT], f32, kind="ExternalOutput")

    with tile.TileContext(nc) as tc:
        with (
            tc.tile_pool(name="const", bufs=1) as cpool,
            tc.tile_pool(name="node", bufs=3) as npool,
            tc.tile_pool(name="npsum", bufs=3, space="PSUM") as npsum,
            tc.tile_pool(name="gth", bufs=4) as gpool,
            tc.tile_pool(name="gth2", bufs=6) as g2pool,
            tc.tile_pool(name="edge", bufs=3) as epool,
            tc.tile_pool(name="accs", bufs=3) as apool,
            tc.tile_pool(name="fin", bufs=2) as fpool,
            tc.tile_pool(name="fpsum", bufs=2, space="PSUM") as fpsum,
        ):
            ident = cpool.tile([128, 128], bf16)
            make_identity(nc, ident[:])
            w1es = cpool.tile([IN, 80], f32)
            nc.sync.dma_start(out=w1es[:], in_=w1e[:])
            b1es = cpool.tile([128, 80], f32)
            nc.sync.dma_start(out=b1es[:], in_=b1e[:])
            w2es = cpool.tile([HID, 4], bf16)
            nc.sync.dma_start(out=w2es[:], in_=w2e[:])
            b2es = cpool.tile([128, 4], bf16)
            nc.sync.dma_start(out=b2es[:], in_=b2e[:])
            pr1s = cpool.tile([88, 320], bf16)
            nc.sync.dma_start(out=pr1s[:], in_=pr1[:])
            pr2s = cpool.tile([1, T2W], bf16)
            nc.sync.dma_start(out=pr2s[:], in_=pr2[:])

            # ---- node phase: table1 = 4-pack [al_src | h+b1 | al_dst]
            # iteration i, partition p, sub j -> node i*512 + 4p + j
            # -> table1 row i*128 + p, elems j*80 .. j*80+80
            for i in range(NITER):
                xti = npool.tile([128, 512], f32, tag="xt")
                nc.scalar.dma_start(out=xti[:], in_=xt[:, i * 512:(i + 1) * 512])
                g1 = npsum.tile([128, 4, 80], f32, tag="g1")
                for j in range(4):
                    nc.tensor.matmul(out=g1[:, j, :],
                                     lhsT=xti[:, j * 128:(j + 1) * 128],
                                     rhs=w1es[:], start=True, stop=True)
                t1s = npool.tile([128, 4, 80], bf16, tag="t1")
                nc.vector.tensor_tensor(
                    out=t1s[:], in0=g1[:],
                    in1=b1es[:].unsqueeze(1).to_broadcast([128, 4, 80]),
                    op=OP.add)
                dst_ap = table1[i * 128:(i + 1) * 128, 0:320].rearrange(
                    "r (j v) -> r j v", v=80)
                nc.sync.dma_start(out=dst_ap, in_=t1s[:])
            # poison rows for pad slots: al_src/al_dst = -1e30, h = 0
            nc.sync.dma_start(
                out=table1[PR1:PR1 + 88, 0:320], in_=pr1s[:])

            def select4(out_ap, gt, j0, kt, voff, nv, oh_t, ew, tag):
                nc.vector.tensor_copy(
                    out=out_ap, in_=gt[:, j0:j0 + kt, voff:voff + nv])
                for i in range(1, 4):
                    nc.vector.copy_predicated(
                        out=out_ap,
                        mask=oh_t[:, j0:j0 + kt, i - 1:i].to_broadcast(
                            [128, kt, nv]),
                        data=gt[:, j0:j0 + kt, i * ew + voff:i * ew + voff + nv])

            # ---- edge phases
            gq = [0]

            def edge_phase(layer):
                if layer == 1:
                    idxd, ohd, tab, EW, EWN, NH, NCH = (
                        idx1d, oh1d, table1, T1W, 80, H1, C1)
                else:
                    idxd, ohd, tab, EW, EWN, NH, NCH = (
                        idx2d, oh2d, table2, T2W, 4, 1, OUT)
                NV = NH + NH * NCH  # 72 / 3
                WR = NH + NH * NCH
                state = {}

                for k in range(nchunks):
                    eng = nc.sync if k % 2 == 0 else nc.scalar
                    idx_t = epool.tile([128, NCALL * IW], i16, tag=f"ix{layer}")
                    eng.dma_start(
                        out=idx_t[:],
                        in_=idxd[:, k * NCALL * IW:(k + 1) * NCALL * IW])
                    oh_t = epool.tile([128, CPC, 3], u8, tag=f"oh{layer}")
                    eng.dma_start(out=oh_t[:],
                                  in_=ohd[:, k * CPC:(k + 1) * CPC, :])
                    pool_g = gpool if layer == 1 else g2pool
                    gt = pool_g.tile([128, CPC, EW], bf16, tag=f"gt{layer}")
                    for q in range(NCALL):
                        nc.gpsimd.dma_gather(
                            gt[:, q * GPC:(q + 1) * GPC, :], tab[:],
                            idx_t[:, q * IW:(q + 1) * IW], NI, NI, EW,
                            queue_num=gq[0] % 4)
                        gq[0] += 1
                    for (t, gc0, gc1, first, last) in segs_of[k]:
                        c0 = gc0 - k * CPC
                        kt = gc1 - gc0
                        V = epool.tile([128, CPC, NV], bf16, tag=f"V{layer}")
                        select4(V[:, 0:kt, :], gt, c0, kt, 0, NV, oh_t, EWN,
                                f"v{layer}")
                        if first:
                            adt = apool.tile([128, 1, NH], bf16,
                                             tag=f"adt{layer}")
                            select4(adt[:], gt, c0, 1, NV, NH, oh_t, EWN,
                                    f"a{layer}")
                            acc = apool.tile([128, WR], f32, tag=f"acc{layer}")
                            state[t] = (adt, acc)
                        else:
                            adt, acc = state[t]
                        eT = epool.tile([128, CPC, NH], f32, tag=f"e{layer}")
                        nc.vector.tensor_tensor(
                            out=eT[:, 0:kt, :], in0=V[:, 0:kt, 0:NH],
                            in1=adt[:].to_broadcast([128, kt, NH]),
                            op=OP.add)
                        lk = epool.tile([128, CPC, NH], f32, tag=f"lk{layer}")
                        nc.vector.scalar_tensor_tensor(
                            out=lk[:, 0:kt, :], in0=eT[:, 0:kt, :], scalar=NEG,
                            in1=eT[:, 0:kt, :], op0=OP.mult, op1=OP.max)
                        W = epool.tile([128, WR, CPC], bf16, tag=f"W{layer}")
                        nc.scalar.activation(
                            out=W[:, 0:NH, 0:kt].rearrange("p h c -> p c h"),
                            in_=lk[:, 0:kt, :], func=ACT.Exp)
                        nc.vector.tensor_tensor(
                            out=W[:, NH:WR, 0:kt].rearrange(
                                "p (h c) j -> p h c j", h=NH),
                            in0=V[:, 0:kt, NH:NH + NH * NCH].rearrange(
                                "p j (h c) -> p h c j", h=NH),
                            in1=W[:, 0:NH, 0:kt].unsqueeze(2).to_broadcast(
                                [128, NH, NCH, kt]),
                            op=OP.mult)
                        if first:
                            nc.vector.tensor_reduce(
                                out=acc[:], in_=W[:, :, 0:kt], axis=AX,
                                op=OP.add)
                        else:
                            red = apool.tile([128, WR], f32, tag=f"red{layer}")
                            nc.vector.tensor_reduce(
                                out=red[:], in_=W[:, :, 0:kt], axis=AX,
                                op=OP.add)
                            nc.vector.tensor_tensor(out=acc[:], in0=acc[:],
                                                    in1=red[:], op=OP.add)
                        if last:
                            finalize(layer, t, acc)
                            del state[t]

            def finalize(layer, t, a):
                NH = H1 if layer == 1 else 1
                NCH = C1 if layer == 1 else OUT
                WR = NH + NH * NCH
                rden = fpool.tile([128, NH], f32, tag=f"rden{layer}")
                nc.vector.reciprocal(out=rden[:], in_=a[:, 0:NH])
                if layer == 1:
                    z = fpool.tile([128, HID], f32, tag="z")
                    nc.vector.tensor_tensor(
                        out=z[:].rearrange("p (h c) -> p h c", h=NH),
                        in0=a[:, NH:WR].rearrange("p (h c) -> p h c", h=NH),
                        in1=rden[:].unsqueeze(2).to_broadcast([128, NH, NCH]),
                        op=OP.mult)
                    # elu -> bf16
                    zm = fpool.tile([128, HID], f32, tag="zm")
                    nc.vector.tensor_scalar(out=zm[:], in0=z[:], scalar1=0.0,
                                            scalar2=None, op0=OP.min)
                    ze = fpool.tile([128, HID], f32, tag="ze")
                    nc.scalar.activation(out=ze[:], in_=zm[:], func=ACT.Exp)
                    nc.vector.tensor_scalar(out=zm[:], in0=z[:], scalar1=0.0,
                                            scalar2=None, op0=OP.max)
                    zb = fpool.tile([128, HID], bf16, tag="zb")
                    nc.vector.scalar_tensor_tensor(
                        out=zb[:], in0=ze[:], scalar=-1.0, in1=zm[:],
                        op0=OP.add, op1=OP.add)
                    # table2 row = z @ W2e + b2e
                    zT_ps = fpsum.tile([HID, 128], bf16, tag="zTp")
                    nc.tensor.transpose(out=zT_ps[:], in_=zb[:],
                                        identity=ident[:])
                    zTs = fpool.tile([HID, 128], bf16, tag="zTs")
                    nc.vector.tensor_copy(out=zTs[:], in_=zT_ps[:])
                    g2 = fpsum.tile([128, 4], f32, tag="g2p")
                    nc.tensor.matmul(out=g2[:], lhsT=zTs[:], rhs=w2es[:],
                                     start=True, stop=True)
                    g2s = fpool.tile([128, 4], bf16, tag="g2s")
                    nc.vector.tensor_tensor(out=g2s[:], in0=g2[:], in1=b2es[:],
                                            op=OP.add)
                    dst_ap = t2shard[t * 32:t * 32 + 32, 0:16].rearrange(
                        "r (n v) -> r n v", v=4)
                    nc.sync.dma_start(out=dst_ap, in_=g2s[:])
                else:
                    o = fpool.tile([128, OUT], f32, tag="o2")
                    nc.vector.tensor_tensor(
                        out=o[:], in0=a[:, 1:1 + OUT],
                        in1=rden[:].to_broadcast([128, OUT]),
                        op=OP.mult)
                    nc.sync.dma_start(
                        out=outp[t * 128:(t + 1) * 128, :], in_=o[:])

            edge_phase(1)

            # ---- exchange layer-2 node features
            nc.gpsimd.collective_compute(
                "AllGather",
                mybir.AluOpType.bypass,
                replica_groups=[list(range(NCORES))],
                ins=[t2shard[:]],
                outs=[table2[:]],
            )
            # poison row for layer-2 pad slots
            nc.sync.dma_start(out=table2[PR2:PR2 + 1, :], in_=pr2s[:])

            edge_phase(2)

    nc.compile()
    return nc


def kernel(**inputs):
    from concourse.bass_utils import run_bass_kernel_spmd

    x = np.asarray(inputs["x"], dtype=np.float32)
    ei = np.asarray(inputs["edge_index"]).astype(np.int64)
    w1 = np.asarray(inputs["W1"], dtype=np.float32)
    a1s = np.asarray(inputs["a1_src"], dtype=np.float32)
    a1d = np.asarray(inputs["a1_dst"], dtype=np.float32)
    b1 = np.asarray(inputs["b1"], dtype=np.float32)
    w2 = np.asarray(inputs["W2"], dtype=np.float32)
    a2s = np.asarray(inputs["a2_src"], dtype=np.float32)
    a2d = np.asarray(inputs["a2_dst"], dtype=np.float32)
    b2 = np.asarray(inputs["b2"], dtype=np.float32)

    src = ei[0]
    dst = ei[1]

    datas, K, col0, nchunks, ncols_pad = _plan(src, dst)

    # permuted global position of each node for the L2 table
    gpos_of_node = np.zeros(NPAD, dtype=np.int64)
    for c in range(NCORES):
        perm = datas[c]["perm"]
        real = perm >= 0
        gpos_of_node[c * ND + perm[real]] = c * PT + np.nonzero(real)[0]

    # weights
    A1s = np.zeros((HID, H1), dtype=np.float32)
    A1d = np.zeros((HID, H1), dtype=np.float32)
    for h in range(H1):
        A1s[h * C1:(h + 1) * C1, h] = a1s[h]
        A1d[h * C1:(h + 1) * C1, h] = a1d[h]
    w1e = np.concatenate([w1 @ A1s, w1, w1 @ A1d], axis=1)      # [128, 80]
    w2e = np.concatenate([w2 @ a2s.T, w2, w2 @ a2d.T], axis=1)  # [64, 4]
    b1e = np.zeros((128, 80), dtype=np.float32)
    b1e[:, H1:H1 + HID] = b1[None, :]
    b2e = np.zeros((128, 4), dtype=BF16)
    b2e[:, 1:1 + OUT] = b2[None, :].astype(BF16)
    # poison rows: per sub-node [al_src(8)=BIG | h(64)=0 | al_dst(8)=BIG]
    sub = np.zeros(80, dtype=np.float32)
    sub[0:H1] = BIG
    sub[H1 + HID:] = BIG
    pr1 = np.tile(sub, (88, 4)).astype(BF16)                    # [88, 320]
    pr2 = np.zeros((1, T2W), dtype=BF16)
    sub2 = np.zeros(4, dtype=np.float32)
    sub2[0] = BIG
    sub2[3] = BIG
    pr2[0, 0:16] = np.tile(sub2, 4).astype(BF16)

    # x transposed + node-phase interleave: xt[c, i*512 + j*128 + p] =
    # x[i*512 + 4p + j, c]
    xp = np.zeros((NPAD, IN), dtype=np.float32)
    xp[:N] = x
    xr = xp.reshape(NITER, 128, 4, IN)            # [i, p, j, c]
    xt = np.ascontiguousarray(xr.transpose(3, 0, 2, 1).reshape(IN, NPAD))

    key = (nchunks, tuple(K.tolist()))
    if key not in _BUILD_CACHE:
        _BUILD_CACHE[key] = _build(K, col0, nchunks)
    nc = _BUILD_CACHE[key]

    common = dict(xt=xt, w1e=w1e, b1e=b1e, w2e=w2e.astype(BF16), b2e=b2e,
                  pr1=pr1, pr2=pr2)
    eye4 = np.eye(4, dtype=np.float32)
    in_maps = []
    for c in range(NCORES):
        node1 = datas[c]["node1"]           # [ncols_pad, 128]
        valid = datas[c]["valid"]
        g = gpos_of_node[node1]

        idx1 = np.where(valid, node1 // 4, PR1).astype(np.int16)
        idx2 = np.where(valid, g // 4, PR2).astype(np.int16)
        s1 = np.where(valid, node1 % 4, 0)
        s2 = np.where(valid, g % 4, 0)
        oh1 = eye4[s1][:, :, 1:].astype(np.uint8)  # [nc,128,3] sub 1..3 masks
        oh2 = eye4[s2][:, :, 1:].astype(np.uint8)

        ngrp = ncols_pad // GPC
        idx1w = np.concatenate(
            [_wrap_idx(idx1[g * GPC:(g + 1) * GPC].reshape(-1))
             for g in range(ngrp)], axis=1)
        idx2w = np.concatenate(
            [_wrap_idx(idx2[g * GPC:(g + 1) * GPC].reshape(-1))
             for g in range(ngrp)], axis=1)

        m = dict(common)
        m["idx1d"] = idx1w
        m["idx2d"] = idx2w
        m["oh1d"] = np.ascontiguousarray(oh1.transpose(1, 0, 2))
        m["oh2d"] = np.ascontiguousarray(oh2.transpose(1, 0, 2))
        in_maps.append(m)

    global _LAST_IN_MAPS
    _LAST_IN_MAPS = in_maps
    res = run_bass_kernel_spmd(nc, in_maps, list(range(NCORES)))

    out = np.zeros((N, OUT), dtype=np.float32)
    for c in range(NCORES):
        op = res.results[c]["outp"]       # [PT, 2] in permuted order
        perm = datas[c]["perm"]
        real = perm >= 0
        out[c * ND + perm[real]] = op[real]
    return out


# revision 26
# speedup vs baseline: 2.2393x; 1.0748x over previous
"""GAT (2-layer, PyG-style) on 8 Trainium2 NeuronCores via Bass/Tile.

Strategy (dst-sharded, degree-sorted tiles, big-batch dma_gather):
- Nodes sharded by dst across 8 cores (12500 each). Per core, dsts are
  degree-sorted and grouped into 98 tiles of 128 (partition = dst).
- Per tile, column 0 gathers the dst's own table row (serving both the
  self-loop edge and the per-partition al_dst values); remaining columns
  hold in-edges, padded to the tile max degree.
- Layer tables are 4-node-packed rows (<=25088 rows, int16 dma_gather
  indices); 4 exclusive predicated copies on DVE pick the node within
  the row (uint8 sub-node masks, no base copy, no one-hot arithmetic).
- Gathers run 1024 indices per call (the SWDGE ring limit) round-robin
  over the 4 SWDGE queues; compute runs on tile-aligned 48-column parts.
- Pad slots index dedicated poison rows (al = -1e30) so exp() kills
  them; no mask-add arrays needed.
- Segment softmax: no max-subtraction needed (logits are O(1)); the
  denominator divides the aggregated numerator once per dst row.
- b1/b2 folded into the h-columns of the tables (alpha sums to 1).
- x is pre-transposed (and bf16-cast) on the host so the node phase is
  matmul-only (no PE transposes); table-1 writes are 640 B contiguous.
- LeakyReLU is one fused scalar_tensor_tensor op; only Exp runs on the
  scalar engine (single activation table, no reloads).
- Layer-2 per-node features (4 values) travel via AllGather of the
  per-core shards in core-local permuted order.
"""

import numpy as np
import ml_dtypes

BF16 = ml_dtypes.bfloat16

N = 100_000
E = 3_200_000
IN = 128
H1, C1 = 8, 8
HID = H1 * C1          # 64
OUT = 2
NEG = 0.2
NCORES = 8
ND = N // NCORES       # dsts per core: 12500
NT = 98                # tiles per core (98*128 = 12544)
PT = NT * 128          # padded dst slots per core: 12544
NPAD = 100_352         # x padded to 784*128
NITER = NPAD // 512    # node-phase iterations (512 nodes each): 196
CPC = 48               # compute-chunk columns (6 gather calls each)
GPC = 8                # columns per dma_gather call (1024 indices)
T1R = NPAD // 4        # 25088 4-pack rows for layer-1 table
T1W = 384              # bf16 elems per table1 row (768 B); 4 x 80 used
T2R = (PT * NCORES) // 4   # 25088
T2W = 128              # bf16 elems per table2 row (256 B); 4 x 4 used
PR1 = NPAD // 4 - 88   # first poison 4-pack row (nodes 100000..100351)
PR2 = T2R - 1          # poison row in table2 (always unused slots)
BIG = -1e30


def _wrap_idx(flat):
    """int16 index array -> [128, n/16] wrapped-in-16-partitions, x8."""
    n = flat.shape[0]
    assert n % 16 == 0
    w = flat.reshape(n // 16, 16).T            # [16, n/16]
    return np.tile(w, (8, 1)).astype(np.int16)  # [128, n/16]


def _plan(src, dst):
    """Host-side index planning. Returns per-core slot arrays + schedule."""
    core = dst // ND
    dloc = dst % ND

    per_core = []
    for c in range(NCORES):
        m = core == c
        s_c = src[m]
        d_c = dloc[m]
        deg = np.bincount(d_c, minlength=ND)  # in-edges, no self loop yet
        order = np.argsort(-deg, kind="stable")  # degree-desc permutation
        perm = np.full(PT, -1, dtype=np.int64)
        perm[:ND] = order
        degp = np.zeros(PT, dtype=np.int64)
        degp[:ND] = deg[order]
        sort_by_d = np.argsort(d_c, kind="stable")
        s_sorted = s_c[sort_by_d]
        starts = np.zeros(ND + 1, dtype=np.int64)
        np.cumsum(deg, out=starts[1:])
        per_core.append(dict(perm=perm, degp=degp, s_sorted=s_sorted, starts=starts))

    # common K_t schedule: columns per tile = 1 (self/dst col) + max in-degree
    K = np.zeros(NT, dtype=np.int64)
    for t in range(NT):
        mx = 0
        for c in range(NCORES):
            d = per_core[c]["degp"][t * 128 : (t + 1) * 128]
            mx = max(mx, int(d.max()) if d.size else 0)
        K[t] = mx + 1
    ncols = int(K.sum())
    nchunks = (ncols + CPC - 1) // CPC
    ncols_pad = nchunks * CPC

    col0 = np.zeros(NT, dtype=np.int64)
    pos = 0
    for t in range(NT):
        col0[t] = pos
        pos += K[t]

    datas = []
    for c in range(NCORES):
        pc = per_core[c]
        perm, degp, s_sorted, starts = (
            pc["perm"], pc["degp"], pc["s_sorted"], pc["starts"],
        )
        node1 = np.zeros((ncols_pad, 128), dtype=np.int64)
        valid = np.zeros((ncols_pad, 128), dtype=bool)
        for t in range(NT):
            base = col0[t]
            d_orig = perm[t * 128 : (t + 1) * 128]
            real = d_orig >= 0
            dg = np.where(real, d_orig, 0)
            # column 0: the dst's own row (self loop + al_dst source)
            node1[base, :] = c * ND + dg
            valid[base, :] = real
            kt = int(K[t])
            if kt > 1:
                st = starts[dg]
                cnt = degp[t * 128 : (t + 1) * 128]
                for j in range(1, kt):
                    sel = (j - 1 < cnt) & real
                    idxs = st + (j - 1)
                    node1[base + j, sel] = s_sorted[np.where(sel, idxs, 0)][sel]
                    valid[base + j, sel] = True
        datas.append(dict(node1=node1, valid=valid, perm=perm))
    return datas, K, col0, nchunks, ncols_pad


_BUILD_CACHE = {}


def _build(K, col0, nchunks):
    import concourse.bass as bass
    import concourse.bacc as bacc
    import concourse.mybir as mybir
    import concourse.tile as tile
    from concourse.masks import make_identity

    f32 = mybir.dt.float32
    bf16 = mybir.dt.bfloat16
    i16 = mybir.dt.int16
    AX = mybir.AxisListType.X
    OP = mybir.AluOpType
    ACT = mybir.ActivationFunctionType

    ncols_pad = nchunks * CPC
    NI = GPC * 128           # indices per gather call: 1024
    IW = NI // 16            # wrapped idx cols per call: 64
    NCALL = CPC // GPC       # gather calls per chunk: 4

    # tile-aligned parts of <= CPC columns; per part, gather calls of
    # <= GPC columns (variable NI). ioffs: running idx-array offset per call.
    parts = []   # (tile, gc0, kt, first, last, callspec)
    ioff = 0
    for t in range(NT):
        a = int(col0[t])
        kt_full = int(K[t])
        for p0 in range(0, kt_full, CPC):
            kt = min(CPC, kt_full - p0)
            calls = []
            c = 0
            while c < kt:
                ncc = min(GPC, kt - c)
                calls.append((c, ncc, ioff))
                ioff += ncc * 8
                c += ncc
            parts.append((t, a + p0, kt, p0 == 0, p0 + kt >= kt_full, calls))
    totw = ioff

    nc = bacc.Bacc("TRN2", target_bir_lowering=False, debug=False,
                   num_devices=NCORES, num_swdge_queues=4)

    xt = nc.dram_tensor("xt", [IN, NPAD], bf16, kind="ExternalInput")
    w1e = nc.dram_tensor("w1e", [IN, 80], bf16, kind="ExternalInput")
    b1e = nc.dram_tensor("b1e", [128, 80], f32, kind="ExternalInput")
    w2e = nc.dram_tensor("w2e", [HID, 4], bf16, kind="ExternalInput")
    b2e = nc.dram_tensor("b2e", [128, 4], bf16, kind="ExternalInput")
    pr1 = nc.dram_tensor("pr1", [88, 320], bf16, kind="ExternalInput")
    pr2 = nc.dram_tensor("pr2", [1, T2W], bf16, kind="ExternalInput")
    idx1d = nc.dram_tensor("idx1d", [128, totw], i16,
                           kind="ExternalInput")
    idx2d = nc.dram_tensor("idx2d", [128, totw], i16,
                           kind="ExternalInput")
    u8 = mybir.dt.uint8
    oh1d = nc.dram_tensor("oh1d", [128, ncols_pad, 4], u8,
                          kind="ExternalInput")
    oh2d = nc.dram_tensor("oh2d", [128, ncols_pad, 4], u8,
                          kind="ExternalInput")

    table1 = nc.dram_tensor("table1", [T1R, T1W], bf16, kind="Internal")
    t2shard = nc.dram_tensor("t2shard", [PT // 4, T2W], bf16, kind="Internal")
    table2 = nc.dram_tensor("table2", [T2R, T2W], bf16, kind="Internal",
                            addr_space="Shared")
    outp = nc.dram_tensor("outp", [PT, OUT], f32, kind="ExternalOutput")

    with tile.TileContext(nc) as tc:
        with (
            tc.tile_pool(name="const", bufs=1) as cpool,
            tc.tile_pool(name="node", bufs=3) as npool,
            tc.tile_pool(name="npsum", bufs=3, space="PSUM") as npsum,
            tc.tile_pool(name="aux", bufs=12) as xpool,
            tc.tile_pool(name="edge", bufs=2) as epool,
            tc.tile_pool(name="accs", bufs=6) as apool,
            tc.tile_pool(name="fin", bufs=2) as fpool,
            tc.tile_pool(name="fpsum", bufs=2, space="PSUM") as fpsum,
        ):
            ident = cpool.tile([128, 128], bf16)
            make_identity(nc, ident[:])
            w1es = cpool.tile([IN, 80], bf16)
            nc.sync.dma_start(out=w1es[:], in_=w1e[:])
            b1es = cpool.tile([128, 80], f32)
            nc.sync.dma_start(out=b1es[:], in_=b1e[:])
            w2es = cpool.tile([HID, 4], bf16)
            nc.sync.dma_start(out=w2es[:], in_=w2e[:])
            b2es = cpool.tile([128, 4], bf16)
            nc.sync.dma_start(out=b2es[:], in_=b2e[:])
            pr1s = cpool.tile([88, 320], bf16)
            nc.sync.dma_start(out=pr1s[:], in_=pr1[:])
            pr2s = cpool.tile([1, T2W], bf16)
            nc.sync.dma_start(out=pr2s[:], in_=pr2[:])

            # ---- node phase: table1 = 4-pack [al_src | h+b1 | al_dst]
            # iteration i, partition p, sub j -> node i*512 + 4p + j
            # -> table1 row i*128 + p, elems j*80 .. j*80+80
            for i in range(NITER):
                xti = npool.tile([128, 512], bf16, tag="xt")
                xeng = nc.scalar if i % 2 == 0 else nc.sync
                xeng.dma_start(out=xti[:], in_=xt[:, i * 512:(i + 1) * 512])
                g1 = npsum.tile([128, 4, 80], f32, tag="g1")
                for j in range(4):
                    nc.tensor.matmul(out=g1[:, j, :],
                                     lhsT=xti[:, j * 128:(j + 1) * 128],
                                     rhs=w1es[:], start=True, stop=True)
                t1s = npool.tile([128, 4, 80], bf16, tag="t1")
                nc.vector.tensor_tensor(
                    out=t1s[:], in0=g1[:],
                    in1=b1es[:].unsqueeze(1).to_broadcast([128, 4, 80]),
                    op=OP.add)
                dst_ap = table1[i * 128:(i + 1) * 128, 0:320].rearrange(
                    "r (j v) -> r j v", v=80)
                nc.sync.dma_start(out=dst_ap, in_=t1s[:])
            # poison rows for pad slots: al_src/al_dst = -1e30, h = 0
            nc.sync.dma_start(
                out=table1[PR1:PR1 + 88, 0:320], in_=pr1s[:])

            def select4(out_ap, gt, j0, kt, voff, nv, oh_t, ew, tag):
                for i in range(4):
                    nc.vector.copy_predicated(
                        out=out_ap,
                        mask=oh_t[:, j0:j0 + kt, i:i + 1].to_broadcast(
                            [128, kt, nv]),
                        data=gt[:, j0:j0 + kt, i * ew + voff:i * ew + voff + nv])

            # ---- edge phases
            gq = [0]

            def edge_phase(layer):
                if layer == 1:
                    idxd, ohd, tab, EW, EWN, NH, NCH = (
                        idx1d, oh1d, table1, T1W, 80, H1, C1)
                else:
                    idxd, ohd, tab, EW, EWN, NH, NCH = (
                        idx2d, oh2d, table2, T2W, 4, 1, OUT)
                NV = EWN            # select the full 80/4-wide sub-row
                WR = NH + NH * NCH

                for pi, (t, gc0, kt, first, last, calls) in enumerate(parts):
                    eng = nc.sync if pi % 2 == 0 else nc.scalar
                    iw = calls[-1][2] + calls[-1][1] * 8 - calls[0][2]
                    i0 = calls[0][2]
                    idx_t = xpool.tile([128, CPC * 8], i16, tag=f"ix{layer}")
                    eng.dma_start(out=idx_t[:, 0:iw],
                                  in_=idxd[:, i0:i0 + iw])
                    oh_t = xpool.tile([128, CPC, 4], u8, tag=f"oh{layer}")
                    eng.dma_start(out=oh_t[:, 0:kt, :],
                                  in_=ohd[:, gc0:gc0 + kt, :])
                    gt = gpools[layer].tile([128, CPC, EW], bf16,
                                            tag=f"gt{layer}")
                    for (c, ncc, io) in calls:
                        nc.gpsimd.dma_gather(
                            gt[:, c:c + ncc, :], tab[:],
                            idx_t[:, io - i0:io - i0 + ncc * 8],
                            ncc * 128, ncc * 128, EW,
                            queue_num=gq[0] % 4)
                        gq[0] += 1
                    c0 = 0
                    if True:
                        V = epool.tile([128, CPC, NV], bf16, tag=f"V{layer}")
                        select4(V[:, 0:kt, :], gt, c0, kt, 0, NV, oh_t, EWN,
                                f"v{layer}")
                        if first:
                            adt = apool.tile([128, 1, NH], bf16,
                                             tag=f"adt{layer}")
                            nc.vector.tensor_copy(
                                out=adt[:], in_=V[:, 0:1, WR:WR + NH])
                            acc = apool.tile([128, WR], f32, tag=f"acc{layer}")
                        eT = epool.tile([128, CPC, NH], bf16, tag=f"e{layer}")
                        nc.vector.tensor_tensor(
                            out=eT[:, 0:kt, :], in0=V[:, 0:kt, 0:NH],
                            in1=adt[:].to_broadcast([128, kt, NH]),
                            op=OP.add)
                        lk = epool.tile([128, CPC, NH], bf16, tag=f"lk{layer}")
                        nc.vector.scalar_tensor_tensor(
                            out=lk[:, 0:kt, :], in0=eT[:, 0:kt, :], scalar=NEG,
                            in1=eT[:, 0:kt, :], op0=OP.mult, op1=OP.max)
                        W = epool.tile([128, WR, CPC], bf16, tag=f"W{layer}")
                        nc.scalar.activation(
                            out=W[:, 0:NH, 0:kt].rearrange("p h c -> p c h"),
                            in_=lk[:, 0:kt, :], func=ACT.Exp)
                        nc.vector.tensor_tensor(
                            out=W[:, NH:WR, 0:kt].rearrange(
                                "p (h c) j -> p h c j", h=NH),
                            in0=V[:, 0:kt, NH:NH + NH * NCH].rearrange(
                                "p j (h c) -> p h c j", h=NH),
                            in1=W[:, 0:NH, 0:kt].unsqueeze(2).to_broadcast(
                                [128, NH, NCH, kt]),
                            op=OP.mult)
                        if first:
                            nc.vector.tensor_reduce(
                                out=acc[:], in_=W[:, :, 0:kt], axis=AX,
                                op=OP.add)
                        else:
                            red = apool.tile([128, WR], f32, tag=f"red{layer}")
                            nc.vector.tensor_reduce(
                                out=red[:], in_=W[:, :, 0:kt], axis=AX,
                                op=OP.add)
                            nc.vector.tensor_tensor(out=acc[:], in0=acc[:],
                                                    in1=red[:], op=OP.add)
                        if last:
                            finalize(layer, t, acc)

            def finalize(layer, t, a):
                NH = H1 if layer == 1 else 1
                NCH = C1 if layer == 1 else OUT
                WR = NH + NH * NCH
                rden = fpool.tile([128, NH], f32, tag=f"rden{layer}")
                nc.vector.reciprocal(out=rden[:], in_=a[:, 0:NH])
                if layer == 1:
                    z = fpool.tile([128, HID], f32, tag="z")
                    nc.vector.tensor_tensor(
                        out=z[:].rearrange("p (h c) -> p h c", h=NH),
                        in0=a[:, NH:WR].rearrange("p (h c) -> p h c", h=NH),
                        in1=rden[:].unsqueeze(2).to_broadcast([128, NH, NCH]),
                        op=OP.mult)
                    # elu -> bf16
                    zm = fpool.tile([128, HID], f32, tag="zm")
                    nc.vector.tensor_scalar(out=zm[:], in0=z[:], scalar1=0.0,
                                            scalar2=None, op0=OP.min)
                    ze = fpool.tile([128, HID], f32, tag="ze")
                    nc.scalar.activation(out=ze[:], in_=zm[:], func=ACT.Exp)
                    nc.vector.tensor_scalar(out=zm[:], in0=z[:], scalar1=0.0,
                                            scalar2=None, op0=OP.max)
                    zb = fpool.tile([128, HID], bf16, tag="zb")
                    nc.vector.scalar_tensor_tensor(
                        out=zb[:], in0=ze[:], scalar=-1.0, in1=zm[:],
                        op0=OP.add, op1=OP.add)
                    # table2 row = z @ W2e + b2e
                    zT_ps = fpsum.tile([HID, 128], bf16, tag="zTp")
                    nc.tensor.transpose(out=zT_ps[:], in_=zb[:],
                                        identity=ident[:])
                    zTs = fpool.tile([HID, 128], bf16, tag="zTs")
                    nc.vector.tensor_copy(out=zTs[:], in_=zT_ps[:])
                    g2 = fpsum.tile([128, 4], f32, tag="g2p")
                    nc.tensor.matmul(out=g2[:], lhsT=zTs[:], rhs=w2es[:],
                                     start=True, stop=True)
                    g2s = fpool.tile([128, 4], bf16, tag="g2s")
                    nc.vector.tensor_tensor(out=g2s[:], in0=g2[:], in1=b2es[:],
                                            op=OP.add)
                    dst_ap = t2shard[t * 32:t * 32 + 32, 0:16].rearrange(
                        "r (n v) -> r n v", v=4)
                    nc.sync.dma_start(out=dst_ap, in_=g2s[:])
                else:
                    o = fpool.tile([128, OUT], f32, tag="o2")
                    nc.vector.tensor_tensor(
                        out=o[:], in0=a[:, 1:1 + OUT],
                        in1=rden[:].to_broadcast([128, OUT]),
                        op=OP.mult)
                    nc.sync.dma_start(
                        out=outp[t * 128:(t + 1) * 128, :], in_=o[:])

            gpools = {}
            with tc.tile_pool(name="gth1", bufs=3) as g1p:
                gpools[1] = g1p
                edge_phase(1)

            # ---- exchange layer-2 node features
            nc.gpsimd.collective_compute(
                "AllGather",
                mybir.AluOpType.bypass,
                replica_groups=[list(range(NCORES))],
                ins=[t2shard[:]],
                outs=[table2[:]],
            )
            # poison row for layer-2 pad slots
            nc.sync.dma_start(out=table2[PR2:PR2 + 1, :], in_=pr2s[:])

            with tc.tile_pool(name="gth2", bufs=6) as g2p:
                gpools[2] = g2p
                edge_phase(2)

    nc.compile()
    return nc


def kernel(**inputs):
    from concourse.bass_utils import run_bass_kernel_spmd

    x = np.asarray(inputs["x"], dtype=np.float32)
    ei = np.asarray(inputs["edge_index"]).astype(np.int64)
    w1 = np.asarray(inputs["W1"], dtype=np.float32)
    a1s = np.asarray(inputs["a1_src"], dtype=np.float32)
    a1d = np.asarray(inputs["a1_dst"], dtype=np.float32)
    b1 = np.asarray(inputs["b1"], dtype=np.float32)
    w2 = np.asarray(inputs["W2"], dtype=np.float32)
    a2s = np.asarray(inputs["a2_src"], dtype=np.float32)
    a2d = np.asarray(inputs["a2_dst"], dtype=np.float32)
    b2 = np.asarray(inputs["b2"], dtype=np.float32)

    src = ei[0]
    dst = ei[1]

    datas, K, col0, nchunks, ncols_pad = _plan(src, dst)

    # permuted global position of each node for the L2 table
    gpos_of_node = np.zeros(NPAD, dtype=np.int64)
    for c in range(NCORES):
        perm = datas[c]["perm"]
        real = perm >= 0
        gpos_of_node[c * ND + perm[real]] = c * PT + np.nonzero(real)[0]

    # weights
    A1s = np.zeros((HID, H1), dtype=np.float32)
    A1d = np.zeros((HID, H1), dtype=np.float32)
    for h in range(H1):
        A1s[h * C1:(h + 1) * C1, h] = a1s[h]
        A1d[h * C1:(h + 1) * C1, h] = a1d[h]
    w1e = np.concatenate([w1 @ A1s, w1, w1 @ A1d], axis=1).astype(BF16)
    w2e = np.concatenate([w2 @ a2s.T, w2, w2 @ a2d.T], axis=1)  # [64, 4]
    b1e = np.zeros((128, 80), dtype=np.float32)
    b1e[:, H1:H1 + HID] = b1[None, :]
    b2e = np.zeros((128, 4), dtype=BF16)
    b2e[:, 1:1 + OUT] = b2[None, :].astype(BF16)
    # poison rows: per sub-node [al_src(8)=BIG | h(64)=0 | al_dst(8)=BIG]
    sub = np.zeros(80, dtype=np.float32)
    sub[0:H1] = BIG
    sub[H1 + HID:] = BIG
    pr1 = np.tile(sub, (88, 4)).astype(BF16)                    # [88, 320]
    pr2 = np.zeros((1, T2W), dtype=BF16)
    sub2 = np.zeros(4, dtype=np.float32)
    sub2[0] = BIG
    sub2[3] = BIG
    pr2[0, 0:16] = np.tile(sub2, 4).astype(BF16)

    # x transposed + node-phase interleave: xt[c, i*512 + j*128 + p] =
    # x[i*512 + 4p + j, c]
    xp = np.zeros((NPAD, IN), dtype=np.float32)
    xp[:N] = x
    xr = xp.reshape(NITER, 128, 4, IN)            # [i, p, j, c]
    xt = np.ascontiguousarray(
        xr.transpose(3, 0, 2, 1).reshape(IN, NPAD)).astype(BF16)

    key = (nchunks, tuple(K.tolist()))
    if key not in _BUILD_CACHE:
        _BUILD_CACHE[key] = _build(K, col0, nchunks)
    nc = _BUILD_CACHE[key]

    common = dict(xt=xt, w1e=w1e, b1e=b1e, w2e=w2e.astype(BF16), b2e=b2e,
                  pr1=pr1, pr2=pr2)
    eye4 = np.eye(4, dtype=np.float32)
    in_maps = []
    for c in range(NCORES):
        node1 = datas[c]["node1"]           # [ncols_pad, 128]
        valid = datas[c]["valid"]
        g = gpos_of_node[node1]

        idx1 = np.where(valid, node1 // 4, PR1).astype(np.int16)
        idx2 = np.where(valid, g // 4, PR2).astype(np.int16)
        s1 = np.where(valid, node1 % 4, 0)
        s2 = np.where(valid, g % 4, 0)
        oh1 = eye4[s1].astype(np.uint8)            # [nc,128,4] sub masks
        oh2 = eye4[s2].astype(np.uint8)

        ngrp = ncols_pad // GPC
        idx1w = np.concatenate(
            [_wrap_idx(idx1[g * GPC:(g + 1) * GPC].reshape(-1))
             for g in range(ngrp)], axis=1)
        idx2w = np.concatenate(
            [_wrap_idx(idx2[g * GPC:(g + 1) * GPC].reshape(-1))
             for g in range(ngrp)], axis=1)

        m = dict(common)
        m["idx1d"] = idx1w
        m["idx2d"] = idx2w
        m["oh1d"] = np.ascontiguousarray(oh1.transpose(1, 0, 2))
        m["oh2d"] = np.ascontiguousarray(oh2.transpose(1, 0, 2))
        in_maps.append(m)

    global _LAST_IN_MAPS
    _LAST_IN_MAPS = in_maps
    res = run_bass_kernel_spmd(nc, in_maps, list(range(NCORES)))

    out = np.zeros((N, OUT), dtype=np.float32)
    for c in range(NCORES):
        op = res.results[c]["outp"]       # [PT, 2] in permuted order
        perm = datas[c]["perm"]
        real = perm >= 0
        out[c * ND + perm[real]] = op[real]
    return out
